# revision 22
# baseline (speedup 1.0000x reference)
"""Grouped GEMM (MoE expert matmul) on 8 TRN2 NeuronCores.

Problem: a [66048, 1024] f32 tokens, b [8, 1024, 1024] f32 expert weights,
static uneven per-expert token counts. d[m] = a[m] @ b[expert(m)].

Strategy (expert-parallel via M-sharding, zero collectives):
- Token rows are assigned host-side to 8 cores x 3 "slots" of (6, 22, 37)
  m-tiles (128 rows each) = 65 tiles/core (= ceil(516/8), the per-core
  floor). Every slot is single-expert; each core receives the 3 expert
  matrices its slots need as inputs b0/b1/b2. The (core,slot)->expert
  binding is pure DATA, so one SPMD program serves all cores. Only 4 of
  520 tiles are zero-padding.
- All operands are pre-packed on the host into PE-ready bf16 layouts:
  a as [tile, ki, kk, m] (already transposed, so the PE runs no
  identity-transpose matmuls) and b as [ki, kk, n]. Per m-tile the PE
  runs exactly 16 accumulating bf16 matmuls (8 K-steps x 2 N-halves)
  into PSUM f32; ScalarE/DVE evict the two halves to bf16 SBUF and the
  result is stored bf16 (rel-err stays ~3e-3 << 2e-2). PE stream is
  gapless at 1 col/cycle after tile 0 - the 221.9us matmul floor.
- b0 is loaded in 3 k-chunks so tile 0's matmuls chase chunk arrivals;
  the last tile's two half-stores ride separate HWDGE engines to trim
  the drain tail. The few dummy warm-up matmuls start the PE p-state
  ramp early AND pin the Tile scheduler's DMA issue order (removing
  them reshuffles loads and costs ~10us - don't).
"""

import numpy as np

GROUP_SIZES = [12288, 10240, 9216, 8192, 7168, 7168, 6144, 5632]
OFFSETS = np.concatenate([[0], np.cumsum(GROUP_SIZES)]).astype(np.int64)
M_TOTAL = int(OFFSETS[-1])  # 65536
K = 1024
N = 1024
E = 8
P = 128
KK = K // P  # 8 k-tiles

# Per-core uniform slot structure, in m-tiles of 128 rows.
SLOT_TILES = (6, 22, 37)  # sum = 65 tiles = 8320 rows per core
TILES_PER_CORE = sum(SLOT_TILES)
ROWS_PER_CORE = TILES_PER_CORE * P
SLOT_ROW_OFF = (0, SLOT_TILES[0] * P, (SLOT_TILES[0] + SLOT_TILES[1]) * P)

# expert id for (slot, core): found by exact-cover search; 4 pad tiles total.
SLOT_EXPERT = (
    (1, 3, 4, 4, 5, 5, 6, 6),  # slot 0: 6 tiles each
    (0, 3, 4, 4, 5, 5, 7, 7),  # slot 1: 22 tiles each
    (0, 0, 1, 1, 2, 2, 3, 6),  # slot 2: 37 tiles each
)

N_WARM = 4  # dummy 128-row matmuls bridging the initial B/A load window


def _build_schedule():
    """Returns list of (core, slot, slot_row_start, global_row_start, nrows)."""
    cursor = [int(OFFSETS[e]) for e in range(E)]
    recs = []
    # Deterministic fill order: slot index, then core.
    for s in range(3):
        for c in range(8):
            e = SLOT_EXPERT[s][c]
            cap = SLOT_TILES[s] * P
            take = min(cap, int(OFFSETS[e + 1]) - cursor[e])
            if take > 0:
                recs.append((c, s, SLOT_ROW_OFF[s], cursor[e], take))
                cursor[e] += take
    for e in range(E):
        assert cursor[e] == int(OFFSETS[e + 1]), (e, cursor[e])
    return recs


_SCHEDULE = _build_schedule()


def _build_bass():
    import concourse.bass as bass  # noqa: F401
    import concourse.mybir as mybir
    import concourse.tile as tile
    from concourse import bacc

    f32 = mybir.dt.float32
    bf16 = mybir.dt.bfloat16

    nc = bacc.Bacc(
        "TRN2", target_bir_lowering=False, debug=False, enable_asserts=False
    )

    # a pre-packed host-side: at[mt, ki, kk, m] = A[mt*128+m, kk*128+ki]
    at = nc.dram_tensor(
        "at", [TILES_PER_CORE, P, KK, P], bf16, kind="ExternalInput"
    ).ap()
    # b pre-packed host-side: bt[ki, kk, n] = B[kk*128+ki, n]
    bs = [
        nc.dram_tensor(f"b{i}", [P, KK, N], bf16, kind="ExternalInput").ap()
        for i in range(3)
    ]
    d = nc.dram_tensor("d", [ROWS_PER_CORE, N], bf16, kind="ExternalOutput").ap()

    NH = 2  # two 512-wide n-halves

    from contextlib import ExitStack

    with tile.TileContext(nc) as tc, ExitStack() as ctx:
        # Dummy matmuls to keep the PE busy (p-state ramp) while b0/a0 load.
        warm = ctx.enter_context(tc.tile_pool(name="warm", bufs=1))
        wsrc = warm.tile([P, P], bf16)
        wps = ctx.enter_context(tc.tile_pool(name="wps", bufs=1, space="PSUM"))
        wp = wps.tile([P, P], f32, name="wp")

        bpool = ctx.enter_context(tc.tile_pool(name="bpool", bufs=1))
        b_sb = [bpool.tile([P, KK, N], bf16, name=f"bsb{i}") for i in range(3)]

        apool = ctx.enter_context(tc.tile_pool(name="apool", bufs=6))
        psd = ctx.enter_context(tc.tile_pool(name="psd", bufs=6, space="PSUM"))
        dpool = ctx.enter_context(tc.tile_pool(name="dpool", bufs=4))

        # which b input each m-tile uses (static, uniform across cores)
        tile_slot = []
        for s in range(3):
            tile_slot += [s] * SLOT_TILES[s]

        def load_tile(m, eng=None):
            a_sb = apool.tile([P, KK, P], bf16, name="a_sb")
            (eng or nc.gpsimd).dma_start(out=a_sb[:], in_=at[m])
            return a_sb

        def mm_tile(m, a_sb):
            bsel = b_sb[tile_slot[m]]
            d_sb = dpool.tile([P, N], bf16, name="d_sb")
            pds = [psd.tile([P, 512], f32, name="pd") for _ in range(NH)]
            for kk in range(KK):
                for nh in range(NH):
                    nc.tensor.matmul(
                        pds[nh][:],
                        a_sb[:, kk, :],
                        bsel[:, kk, nh * 512 : (nh + 1) * 512],
                        start=(kk == 0),
                        stop=(kk == KK - 1),
                    )
            nc.scalar.copy(d_sb[:, 0:512], pds[0][:])
            if m == TILES_PER_CORE - 1:
                # split the final store across both HWDGE engines so the
                # drain tail is a single 512-half behind one DGE chain
                nc.scalar.dma_start(
                    out=d[m * P : (m + 1) * P, 0:512], in_=d_sb[:, 0:512]
                )
                nc.vector.tensor_copy(d_sb[:, 512:1024], pds[1][:])
                nc.sync.dma_start(
                    out=d[m * P : (m + 1) * P, 512:1024], in_=d_sb[:, 512:1024]
                )
            else:
                nc.vector.tensor_copy(d_sb[:, 512:1024], pds[1][:])
                nc.sync.dma_start(out=d[m * P : (m + 1) * P, :], in_=d_sb[:])

        LOAD_AHEAD = 4
        a_sbs = {}
        # a0 goes via SP HWDGE so its transfer front-runs the Pool SWDGE
        # chain; b0 is split k-wise so tile 0's first matmuls start early.
        a_sbs[0] = load_tile(0, eng=nc.sync)
        nc.gpsimd.dma_start(out=b_sb[0][:, 0:3, :], in_=bs[0][:, 0:3, :])
        nc.gpsimd.dma_start(out=b_sb[0][:, 3:6, :], in_=bs[0][:, 3:6, :])
        nc.gpsimd.dma_start(out=b_sb[0][:, 6:8, :], in_=bs[0][:, 6:8, :])
        nc.vector.memset(wsrc[:], 0.0)
        for _ in range(N_WARM):
            nc.tensor.matmul(wp[:], wsrc[:], wsrc[:], start=True, stop=True)
        for m in range(1, LOAD_AHEAD + 1):
            a_sbs[m] = load_tile(m)
        nc.gpsimd.dma_start(out=b_sb[1][:], in_=bs[1])
        for m in range(TILES_PER_CORE):
            if m == 6:
                nc.gpsimd.dma_start(out=b_sb[2][:], in_=bs[2])
            if m + LOAD_AHEAD + 1 < TILES_PER_CORE:
                a_sbs[m + LOAD_AHEAD + 1] = load_tile(m + LOAD_AHEAD + 1)
            mm_tile(m, a_sbs.pop(m))

    nc.compile()
    return nc


_NC_CACHE = None


def kernel(a, b):
    global _NC_CACHE
    import ml_dtypes
    from concourse.bass_utils import run_bass_kernel_spmd

    bf16 = ml_dtypes.bfloat16

    a = np.ascontiguousarray(np.asarray(a), dtype=np.float32)
    b = np.ascontiguousarray(np.asarray(b), dtype=np.float32)
    assert a.shape == (M_TOTAL, K), a.shape
    assert b.shape == (E, K, N), b.shape

    if _NC_CACHE is None:
        _NC_CACHE = _build_bass()
    nc = _NC_CACHE

    a_bf = a.astype(bf16)
    # Pack per-core A shards into the PE-ready transposed tile layout:
    # at[mt, ki, kk, m] = A_shard[mt*128 + m, kk*128 + ki]
    at_shards = []
    for c in range(8):
        shard = np.zeros((ROWS_PER_CORE, K), dtype=bf16)
        for cc, s, soff, goff, n in _SCHEDULE:
            if cc == c:
                shard[soff : soff + n] = a_bf[goff : goff + n]
        at_shards.append(
            np.ascontiguousarray(
                shard.reshape(TILES_PER_CORE, P, KK, P).transpose(0, 3, 2, 1)
            )
        )

    # Pack each expert's B once: bt[ki, kk, n] = B_e[kk*128 + ki, n]
    b_bf = b.astype(bf16)
    b_packed = [
        np.ascontiguousarray(b_bf[e].reshape(KK, P, N).transpose(1, 0, 2))
        for e in range(E)
    ]

    in_maps = []
    for c in range(8):
        m = {"at": at_shards[c]}
        for s in range(3):
            m[f"b{s}"] = b_packed[SLOT_EXPERT[s][c]]
        in_maps.append(m)

    res = run_bass_kernel_spmd(nc, in_maps, core_ids=list(range(8)))

    out = np.empty((M_TOTAL, N), dtype=np.float32)
    for c, s, soff, goff, n in _SCHEDULE:
        out[goff : goff + n] = res.results[c]["d"][soff : soff + n].astype(
            np.float32
        )
    return out
